# revision 58
# baseline (speedup 1.0000x reference)
"""Trainium2 Bass kernel for nn_MultiHeadAttention_KT (causal linear attention).

Math (per batch b):
  q' = leaky((q*qm) @ Wq + bq); k' = leaky((k*km) @ Wk + bk); v' = (v*vm) @ Wv
  per head h (DEPTH=64):   S_t = sum_{s<=t} k_s v_s^T ; z_t = sum_{s<=t} k_s
                           attn_t = (q_t @ S_t) / (q_t . z_t)
  out = concat_heads(attn) @ Wo + bo

Sharding: 8 cores = 2 batches x 4 head-groups (4 heads / 256 cols each).
Host transposes + bf16-casts inputs (xq = bf16((q*qm)^T) etc.); host sums the
4 partial output projections per batch.

v2 design (vs baseline): all matmul operands bf16 (1 cyc/row on PE vs fp32's
4-pass mode), heads processed in pairs per 128-partition block:
  - AT = K Q^T per head [s,t], masked w/ triu -> bf16 (one [128,256] tile / pair)
  - num2[t, 0:65|65:130] = ATm^T [V|1] per head + Q_pair @ Saug2 (one matmul for
    both heads via block-diagonal Saug2 in SBUF)
  - state: Saug2 accumulates IN PSUM via matmul start=False; off-diagonal junk
    (knatA^T vtB) is never read; SBUF copy is block-diag-masked.
  - phases interleaved (proj sc -> attn chunks of sc-1 -> out-proj of sc-2) so
    the PE never drains and elementwise work spreads across DVE/Act/Pool.
"""

import os
import sys

sys.path.insert(0, "/opt/trn_rl_repo")

import numpy as np

B, S, D, H = 2, 2048, 1024, 16
DEPTH = 64
N_CORES = 8
HPC = 4                 # heads per core
JS = HPC * DEPTH        # 256 projected columns per core
C = 128                 # attention chunk length
NCH = S // C            # 16 chunks
IB = D // 128           # 8 contraction blocks
SC5 = 512               # projection s-chunk
NSC5 = S // SC5         # 4 projection chunks
JA = DEPTH + 1          # 65 ([V | 1] augmented width per head)
JA2 = 2 * JA            # 130 (two heads packed)

MM_DTYPE = "bf16"
TRACE = False           # set True from test harness to capture NTFF profile
TRACE_CORES = None
LAST_RESULTS = None     # BassKernelResults of the last kernel() call

_PROG = None


def _build():
    import concourse.bacc as bacc
    import concourse.mybir as mybir
    import concourse.tile as tile

    dt = mybir.dt
    f32 = dt.float32
    bf16 = dt.bfloat16
    AF = mybir.ActivationFunctionType
    Alu = mybir.AluOpType

    nc = bacc.Bacc("TRN2", target_bir_lowering=False, debug=False,
                   num_devices=N_CORES)

    xq = nc.dram_tensor("xq", [D, S], bf16, kind="ExternalInput").ap()
    xk = nc.dram_tensor("xk", [D, S], bf16, kind="ExternalInput").ap()
    xv = nc.dram_tensor("xv", [D, S], bf16, kind="ExternalInput").ap()
    wq = nc.dram_tensor("wq", [D, JS], bf16, kind="ExternalInput").ap()
    wk = nc.dram_tensor("wk", [D, JS], bf16, kind="ExternalInput").ap()
    wv = nc.dram_tensor("wv", [D, JS], bf16, kind="ExternalInput").ap()
    wo = nc.dram_tensor("wo", [JS, D], bf16, kind="ExternalInput").ap()
    bqd = nc.dram_tensor("bq", [2, 128], f32, kind="ExternalInput").ap()
    bkd = nc.dram_tensor("bk", [2, 128], f32, kind="ExternalInput").ap()
    triu = nc.dram_tensor("triu", [128, 256], f32, kind="ExternalInput").ap()
    ident = nc.dram_tensor("ident", [128, 128], bf16, kind="ExternalInput").ap()
    po = nc.dram_tensor("po", [D, S], bf16, kind="ExternalOutput").ap()
    debug = os.environ.get("KT_DEBUG") == "1"
    if debug:
        qTo = nc.dram_tensor("qTo", [128, 2, S], bf16, kind="ExternalOutput").ap()
        kTo = nc.dram_tensor("kTo", [128, 2, S], bf16, kind="ExternalOutput").ap()
        vauo = nc.dram_tensor("vauo", [128, NCH, 2, JA2], bf16,
                              kind="ExternalOutput").ap()
        aTo = nc.dram_tensor("aTo", [128, 2, S], bf16, kind="ExternalOutput").ap()
        sauo = nc.dram_tensor("sauo", [NCH, 128, 2, JA2], bf16,
                              kind="ExternalOutput").ap()

    mm = nc.tensor.matmul

    xq_r = xq.rearrange("(ib p) s -> p ib s", p=128)
    xk_r = xk.rearrange("(ib p) s -> p ib s", p=128)
    xv_r = xv.rearrange("(ib p) s -> p ib s", p=128)
    po_r = po.rearrange("(ob p) s -> ob p s", p=128)

    with tile.TileContext(nc) as tc:
        with (
            tc.tile_pool(name="persist", bufs=1) as pp,
            tc.tile_pool(name="xin", bufs=2) as xpool,
            tc.tile_pool(name="work", bufs=3) as wkp,
            tc.tile_pool(name="outp", bufs=3) as opool,
            tc.tile_pool(name="p512", bufs=2, space="PSUM") as p512,
            tc.tile_pool(name="p256", bufs=2, space="PSUM") as p256,
            tc.tile_pool(name="pnum", bufs=2, space="PSUM") as pnum,
            tc.tile_pool(name="ptr", bufs=1, space="PSUM") as ptr,
            tc.tile_pool(name="psS", bufs=1, space="PSUM") as psSp,
        ):
            # ---- persistent SBUF ------------------------------------------
            wq_sb = pp.tile([128, IB, JS], bf16, tag="wq", name="wq_sb")
            wk_sb = pp.tile([128, IB, JS], bf16, tag="wk", name="wk_sb")
            wv_sb = pp.tile([128, IB, JS], bf16, tag="wv", name="wv_sb")
            wo_sb = pp.tile([128, 2, D], bf16, tag="wo", name="wo_sb")
            bq_sb = pp.tile([128, 2], f32, tag="bq", name="bq_sb")
            bk_sb = pp.tile([128, 2], f32, tag="bk", name="bk_sb")
            triu_sb = pp.tile([128, 256], f32, tag="triu", name="triu_sb")
            ident_sb = pp.tile([128, 128], bf16, tag="ident", name="ident_sb")
            qT_sb = pp.tile([128, 2, S], bf16, tag="qT", name="qT_sb")
            kT_sb = pp.tile([128, 2, S], bf16, tag="kT", name="kT_sb")
            aT_sb = pp.tile([128, 2, S], bf16, tag="aT", name="aT_sb")
            vaug_sb = pp.tile([128, NCH, 2, JA2], bf16, tag="vaug",
                              name="vaug_sb")
            saug_sb = pp.tile([128, 2, JA2], bf16, tag="saug", name="saug_sb")

            # persistent PSUM state accumulator (junk-tolerant off-diagonal)
            psS = psSp.tile([128, 2, JA2], f32, tag="S", name="psS")

            # ---- phase 0: the three projection weights first, one per DMA
            # ring, so the first q/k/v matmul chains unblock ASAP. wo and the
            # small constants are issued after the first x chunk (emit_smalls)
            # q-path gates the first matmul: its weight and its x chunk go on
            # DIFFERENT HWDGE rings so they transfer in parallel (startup DMA
            # runs well below nominal bandwidth with all 8 cores pulling).
            nc.sync.dma_start(wq_sb[:], wq.rearrange("(ib p) j -> p ib j", p=128))
            nc.sync.dma_start(wk_sb[:], wk.rearrange("(ib p) j -> p ib j", p=128))
            nc.sync.dma_start(wv_sb[:], wv.rearrange("(ib p) j -> p ib j", p=128))
            nc.vector.memset(saug_sb[:], 0.0)
            # ones-columns of [V | 1] for every chunk/head, written once
            nc.vector.memset(
                vaug_sb[:].rearrange("p a b (h e) -> p a b h e", e=JA)
                [:, :, :, :, DEPTH:JA], 1.0)

            xt = {}

            def emit_loads(sc5):
                s0 = sc5 * SC5
                xq_t = xpool.tile([128, IB, SC5], bf16, tag="xq")
                xk_t = xpool.tile([128, IB, SC5], bf16, tag="xk")
                xv_t = xpool.tile([128, IB, SC5], bf16, tag="xv")
                # sc5==0 startup: x chunks on the scalar HWDGE ring, in
                # parallel with the weights on sync (gpsimd SWDGE is slow)
                if sc5 == 0:
                    nc.scalar.dma_start(xq_t[:], xq_r[:, :, s0:s0 + SC5])
                    nc.scalar.dma_start(xk_t[:], xk_r[:, :, s0:s0 + SC5])
                    nc.scalar.dma_start(xv_t[:], xv_r[:, :, s0:s0 + SC5])
                else:
                    nc.sync.dma_start(xq_t[:], xq_r[:, :, s0:s0 + SC5])
                    nc.gpsimd.dma_start(xk_t[:], xk_r[:, :, s0:s0 + SC5])
                    nc.gpsimd.dma_start(xv_t[:], xv_r[:, :, s0:s0 + SC5])
                xt[sc5] = (xq_t, xk_t, xv_t)

            def emit_smalls():
                nc.sync.dma_start(bq_sb[:], bqd.rearrange("jb p -> p jb"))
                nc.sync.dma_start(bk_sb[:], bkd.rearrange("jb p -> p jb"))
                nc.sync.dma_start(triu_sb[:], triu)
                nc.sync.dma_start(ident_sb[:], ident)
                nc.gpsimd.dma_start(
                    wo_sb[:], wo.rearrange("(jb p) o -> p jb o", p=128))

            def emit_proj(sc5):
                s0 = sc5 * SC5
                xq_t, xk_t, xv_t = xt.pop(sc5)
                # q'/k' transposed: psum [128 j, 512 s]
                for x_t, w_sb, b_sb, dst in (
                    (xq_t, wq_sb, bq_sb, qT_sb),
                    (xk_t, wk_sb, bk_sb, kT_sb),
                ):
                    for jb in range(2):
                        ps = p512.tile([128, SC5], f32, tag="p512")
                        for ib in range(IB):
                            mm(ps[:], w_sb[:, ib, jb * 128:(jb + 1) * 128],
                               x_t[:, ib, :],
                               start=(ib == 0), stop=(ib == IB - 1))
                        nc.scalar.activation(
                            dst[:, jb, s0:s0 + SC5], ps[:], AF.Prelu,
                            bias=b_sb[:, jb:jb + 1], scale=1.0, alpha=0.1)
                # v' natural: psum [128 s, 256 j] per 128-block
                for ss in range(SC5 // 128):
                    ci = sc5 * 4 + ss
                    ps = p256.tile([128, JS], f32, tag="p256")
                    for ib in range(IB):
                        mm(ps[:], xv_t[:, ib, ss * 128:(ss + 1) * 128],
                           wv_sb[:, ib, :],
                           start=(ib == 0), stop=(ib == IB - 1))
                    for jb in range(2):
                        src = ps[:, jb * 128:(jb + 1) * 128].rearrange(
                            "p (h e) -> p h e", h=2)
                        dst = vaug_sb[:, ci, jb, :].rearrange(
                            "p (h e) -> p h e", h=2)
                        nc.scalar.activation(dst[:, :, 0:DEPTH], src, AF.Copy)

            def emit_attn_transpose(ci, jbs=(0, 1)):
                # attn^T of chunk ci (emitted lagged, in chunk ci+1's slot)
                scol = ci * C
                for jb in jbs:
                    # transpose as a regular matmul (attn2^T @ I) so PSUM
                    # stays f32 (bf16-in-PSUM is untrusted on hw)
                    tp = ptr.tile([128, 128], f32, tag="tr")
                    mm(tp[:], attn_tiles.pop((ci, jb))[:], ident_sb[:],
                       start=True, stop=True)
                    nc.vector.tensor_copy(aT_sb[:, jb, scol:scol + C], tp[:])

            attn_tiles = {}
            knat_tiles = {}
            use_state = os.environ.get("KT_NO_STATE") != "1"

            def emit_attn(ci):
                scol = ci * C
                # PE-order: prev attn transposes + knat transposes + AT mms
                # first (their DVE/Act consumers run while PE streams on), then
                # num/delta matmuls.
                for jb in range(2):
                    kT2 = kT_sb[:, jb, scol:scol + C]
                    qT2 = qT_sb[:, jb, scol:scol + C]

                    if ci < NCH - 1 and use_state:
                        # knat2 [t, d2] for both heads: kT2^T @ I as a
                        # regular matmul (f32 PSUM out)
                        tp = ptr.tile([128, 128], f32, tag="tr")
                        mm(tp[:], kT2, ident_sb[:], start=True, stop=True)
                        kn = wkp.tile([128, 128], bf16, tag="knat")
                        nc.scalar.activation(kn[:], tp[:], AF.Copy)
                        knat_tiles[(ci, jb)] = kn

                    # AT = K Q^T per head [s, t]; each head's AT in its own
                    # PSUM bank so every matmul is a clean start/stop group.
                    atm = wkp.tile([128, 256], bf16, tag="atm")
                    for hh in range(2):
                        at = p256.tile([128, C], f32, tag="p256")
                        mm(at[:], kT2[hh * DEPTH:(hh + 1) * DEPTH, :],
                           qT2[hh * DEPTH:(hh + 1) * DEPTH, :],
                           start=True, stop=True)
                        nc.vector.tensor_tensor(atm[:, hh * C:(hh + 1) * C],
                                                at[:], triu_sb[:, 0:C],
                                                op=Alu.mult)
                    attn_tiles[(ci, jb, "atm")] = atm

                    if ci > 0:
                        emit_attn_transpose(ci - 1, jbs=(jb,))

                for jb in range(2):
                    qT2 = qT_sb[:, jb, scol:scol + C]
                    atm = attn_tiles.pop((ci, jb, "atm"))
                    va = vaug_sb[:, ci, jb, :]

                    # num2 [t, 130]: history (both heads, block-diag saug) +
                    # per-head diagonal-block terms
                    # num2 as ONE uniform accumulation group per bank: the
                    # history matmul always runs (saug is zeros at ci=0).
                    nm = pnum.tile([128, JA2], f32, tag="num")
                    mm(nm[:], qT2, saug_sb[:, jb, :],
                       start=True, stop=False, skip_group_check=True)
                    mm(nm[:, 0:JA], atm[:, 0:C], va[:, 0:JA],
                       start=False, stop=False, skip_group_check=True)
                    mm(nm[:, JA:JA2], atm[:, C:2 * C], va[:, JA:JA2],
                       start=False, stop=True, skip_group_check=True)

                    rc = wkp.tile([128, 2], f32, tag="rc")
                    nc.vector.reciprocal(rc[:, 0:1], nm[:, DEPTH:DEPTH + 1])
                    nc.vector.reciprocal(rc[:, 1:2], nm[:, JA + DEPTH:JA + DEPTH + 1])
                    at2 = wkp.tile([128, 128], bf16, tag="attn2")
                    nc.scalar.activation(at2[:, 0:DEPTH], nm[:, 0:DEPTH],
                                         AF.Copy, scale=rc[:, 0:1])
                    nc.scalar.activation(at2[:, DEPTH:128], nm[:, JA:JA + DEPTH],
                                         AF.Copy, scale=rc[:, 1:2])
                    attn_tiles[(ci, jb)] = at2

                    # state update in PSUM (junk-tolerant), masked copy to SBUF
                    if ci < NCH - 1 and use_state:
                        kn = knat_tiles.pop((ci, jb))
                        mm(psS[:, jb, :], kn[:], va,
                           start=(ci == 0 and jb == 0), stop=True,
                           skip_group_check=True)
                        nc.vector.tensor_copy(saug_sb[0:DEPTH, jb, 0:JA],
                                              psS[0:DEPTH, jb, 0:JA])
                        nc.vector.tensor_copy(saug_sb[DEPTH:128, jb, JA:JA2],
                                              psS[DEPTH:128, jb, JA:JA2])
                if debug and ci < NCH - 1:
                    nc.sync.dma_start(sauo[ci], saug_sb[:])

            def emit_ph3(sq, obs=None):
                # output projection for s-cols [sq*512, (sq+1)*512)
                for ob in (obs if obs is not None else range(D // 128)):
                    ps = p512.tile([128, SC5], f32, tag="p512")
                    for jb in range(2):
                        mm(ps[:], wo_sb[:, jb, ob * 128:(ob + 1) * 128],
                           aT_sb[:, jb, sq * SC5:(sq + 1) * SC5],
                           start=(jb == 0), stop=(jb == 1))
                    ot = opool.tile([128, SC5], bf16, tag="ot")
                    if (ob + sq) % 2 == 0:
                        nc.scalar.activation(ot[:], ps[:], AF.Copy)
                    else:
                        nc.vector.tensor_copy(ot[:], ps[:])
                    nc.sync.dma_start(po_r[ob, :, sq * SC5:(sq + 1) * SC5],
                                      ot[:])

            # ---- schedule (env flags for hw bisection) --------------------
            skip_attn = os.environ.get("KT_SKIP_ATTN") == "1"
            seq = os.environ.get("KT_SEQ") == "1"
            if skip_attn:
                nc.vector.memset(aT_sb[:], 0.0)
            if seq:
                emit_loads(0)
                emit_smalls()
                for sc5 in range(NSC5):
                    if sc5 + 1 < NSC5:
                        emit_loads(sc5 + 1)
                    emit_proj(sc5)
                if not skip_attn:
                    for ci in range(NCH):
                        emit_attn(ci)
                    emit_attn_transpose(NCH - 1)
                for sq in range(NSC5):
                    emit_ph3(sq)
            else:
                emit_loads(0)
                emit_smalls()
                for sc5 in range(NSC5):
                    if sc5 + 1 < NSC5:
                        emit_loads(sc5 + 1)
                    emit_proj(sc5)
                    if sc5 >= 1 and not skip_attn:
                        for ss in range(4):
                            emit_attn(4 * (sc5 - 1) + ss)
                            # slot out-proj PE work between attention chunks
                            if sc5 >= 2:
                                emit_ph3(sc5 - 2, obs=range(2 * ss, 2 * ss + 2))
                if not skip_attn:
                    # tail: keep the PE fed between the state-serial chunks
                    emit_attn(12)
                    emit_attn(13)
                    emit_ph3(NSC5 - 2, obs=range(0, 4))
                    emit_attn(14)
                    emit_attn(15)
                    emit_ph3(NSC5 - 2, obs=range(4, 8))
                    emit_attn_transpose(NCH - 1)
                    emit_ph3(NSC5 - 1)
                else:
                    for sq in range(NSC5):
                        emit_ph3(sq)
            if debug:
                nc.sync.dma_start(qTo, qT_sb[:])
                nc.sync.dma_start(kTo, kT_sb[:])
                nc.sync.dma_start(vauo, vaug_sb[:])
                nc.sync.dma_start(aTo, aT_sb[:])

    nc.compile()
    return nc


def _get_prog():
    global _PROG
    if _PROG is None:
        _PROG = _build()
    return _PROG


def kernel(q, k, v, query_mask, key_mask, value_mask,
           Wq, bq, Wk, bk, Wv, bv, Wo, bo):
    global LAST_RESULTS
    import ml_dtypes
    from concourse import bass_utils

    bf16 = ml_dtypes.bfloat16

    q = np.asarray(q, np.float32)
    k = np.asarray(k, np.float32)
    v = np.asarray(v, np.float32)
    qm = q * np.asarray(query_mask, np.float32)
    km = k * np.asarray(key_mask, np.float32)
    vm = v * np.asarray(value_mask, np.float32)
    Wq = np.asarray(Wq, np.float32)
    Wk = np.asarray(Wk, np.float32)
    Wv = np.asarray(Wv, np.float32)
    Wo = np.asarray(Wo, np.float32)
    bq = np.asarray(bq, np.float32)
    bk = np.asarray(bk, np.float32)
    bv = np.asarray(bv, np.float32)
    bo = np.asarray(bo, np.float32)
    assert not np.any(bv), "kernel assumes bv == 0 (true for this problem)"

    nc = _get_prog()

    triu2 = np.tile(np.triu(np.ones((128, 128), np.float32)), (1, 2))
    ident = np.eye(128, dtype=bf16)
    xqs = [np.ascontiguousarray(qm[b].T).astype(bf16) for b in range(B)]
    xks = [np.ascontiguousarray(km[b].T).astype(bf16) for b in range(B)]
    xvs = [np.ascontiguousarray(vm[b].T).astype(bf16) for b in range(B)]

    in_maps = []
    for c in range(N_CORES):
        b, g = divmod(c, HPC)
        js = slice(g * JS, (g + 1) * JS)
        in_maps.append({
            "xq": xqs[b], "xk": xks[b], "xv": xvs[b],
            "wq": np.ascontiguousarray(Wq[:, js]).astype(bf16),
            "wk": np.ascontiguousarray(Wk[:, js]).astype(bf16),
            "wv": np.ascontiguousarray(Wv[:, js]).astype(bf16),
            "wo": np.ascontiguousarray(Wo[js, :]).astype(bf16),
            "bq": np.ascontiguousarray(bq[js].reshape(2, 128)),
            "bk": np.ascontiguousarray(bk[js].reshape(2, 128)),
            "triu": triu2, "ident": ident,
        })

    res = bass_utils.run_bass_kernel_spmd(
        nc, in_maps, core_ids=list(range(N_CORES)),
        trace=TRACE, trace_cores=TRACE_CORES)
    LAST_RESULTS = res

    out = np.zeros((B, S, D), np.float32)
    for c in range(N_CORES):
        out[c // HPC] += np.asarray(res.results[c]["po"], np.float32).T
    out += bo
    return out
